# revision 20
# baseline (speedup 1.0000x reference)
"""Trainium2 Bass kernel for the shifted-window attention block (Swin-style).

Sharding: data-parallel over batch B=16 across 8 NeuronCores (2 images/core).
No collectives needed; each core runs the full block on its shard.

Host preprocessing touches only weight-like inputs (LN gamma/beta folding,
attention scale folding, rel-pos-bias + shift-mask table, bf16 casts).
All x-dependent compute happens on device.
"""

import os
import sys

import numpy as np

for _p in (
    "/opt/trn_rl_repo",
    os.path.expanduser("~/.axon_site/_ro/trn_rl_repo"),
):
    if os.path.isdir(_p) and _p not in sys.path:
        sys.path.append(_p)

import ml_dtypes  # noqa: E402

import concourse.bass as bass  # noqa: E402
import concourse.bacc as bacc  # noqa: E402
import concourse.mybir as mybir  # noqa: E402
import concourse.tile as tile  # noqa: E402

# ---------------- problem constants (hardcoded per spec) ----------------
B, H, W, C = 16, 64, 64, 192
NH, HD, P = 6, 32, 8          # heads, head_dim, window size
SHIFT = P // 2                # 4
NWS = (H // P) * (W // P)     # 64 windows per image
PP = P * P                    # 64 tokens per window
EPS = 1e-5
NCORES = 8
BPC = B // NCORES             # 2 images per core
TOK = BPC * H * W             # 8192 tokens per core
CH = 512                      # tokens per chunk = 8 windows = one (b, wi) row
NCH = TOK // CH               # 16 chunks
MASK_NEG = -30000.0

F32 = mybir.dt.float32
BF16 = mybir.dt.bfloat16
AF = mybir.ActivationFunctionType
ALU = mybir.AluOpType

_CACHE = {}


# ======================================================================
# host-side constant prep (weights only -- nothing depends on x)
# ======================================================================
def _shift_mask_types():
    """4 mask types [4, PP, PP] (q, k): 0 interior, 1 bottom (wi==7),
    2 right (wj==7), 3 corner."""
    s = P - SHIFT
    pi = np.arange(P)
    row_half = (pi < s)  # first s rows come from original rows, rest wrapped
    # bottom mask: (pi<s) XOR (ki<s) over the row index of the token
    qi = np.repeat(row_half, P)          # token -> pi<s, row-major (pi*8+pj)
    qj = np.tile(row_half, P)            # token -> pj<s
    m_bot = qi[:, None] != qi[None, :]
    m_rgt = qj[:, None] != qj[None, :]
    return np.stack(
        [np.zeros_like(m_bot), m_bot, m_rgt, m_bot | m_rgt], 0
    )  # [4, PP, PP]


def _rel_bias_np(rel_pos):
    coords = np.stack(
        np.meshgrid(np.arange(P), np.arange(P), indexing="ij"), -1
    ).reshape(-1, 2)
    rel = coords[:, None] - coords[None, :] + P - 1
    return rel_pos[:, rel[..., 0], rel[..., 1]]  # [NH, PP(q), PP(k)]


def _host_prep(inputs):
    f32 = np.float32
    g1 = np.asarray(inputs["g1"], f32)
    b1 = np.asarray(inputs["b1"], f32)
    g2 = np.asarray(inputs["g2"], f32)
    b2 = np.asarray(inputs["b2"], f32)
    w_qkv = np.asarray(inputs["w_qkv"], f32)
    b_qkv = np.asarray(inputs["b_qkv"], f32)
    w_proj = np.asarray(inputs["w_proj"], f32)
    b_proj = np.asarray(inputs["b_proj"], f32)
    w_mlp1 = np.asarray(inputs["w_mlp1"], f32)
    b_mlp1 = np.asarray(inputs["b_mlp1"], f32)
    w_mlp2 = np.asarray(inputs["w_mlp2"], f32)
    b_mlp2 = np.asarray(inputs["b_mlp2"], f32)
    rel_pos = np.asarray(inputs["rel_pos"], f32)

    scale = HD ** -0.5
    w1 = g1[:, None] * w_qkv
    bb1 = b_qkv + b1 @ w_qkv
    w1[:, :C] *= scale
    bb1[:C] *= scale

    w3 = g2[:, None] * w_mlp1
    bb3 = b_mlp1 + b2 @ w_mlp1

    # bias+mask table: bm[k, wtype, h*PP+q] = rel_bias[h, q, k] + MASK_NEG*mask[t, q, k]
    rb = _rel_bias_np(rel_pos)                       # [NH, q, k]
    mtypes = _shift_mask_types().astype(f32) * MASK_NEG  # [4, q, k]
    bm = np.zeros((PP, 4, NH * PP), f32)
    for t in range(4):
        for h in range(NH):
            bm[:, t, h * PP:(h + 1) * PP] = (rb[h] + mtypes[t]).T

    bf = ml_dtypes.bfloat16
    return {
        "w1": w1.astype(bf), "bb1": bb1.reshape(1, -1).astype(bf),
        "wp": w_proj.astype(bf), "bbp": b_proj.reshape(1, -1).astype(bf),
        "w3": w3.astype(bf), "bb3": bb3.reshape(1, -1).astype(bf),
        "w4": w_mlp2.astype(bf), "bb4": b_mlp2.reshape(1, -1).astype(bf),
        "bm": bm.astype(bf),
        "ident": np.eye(PP, dtype=bf),
        "bias_nz": (
            np.any(bb1 != 0), np.any(b_proj != 0),
            np.any(bb3 != 0), np.any(b_mlp2 != 0),
        ),
    }


# ======================================================================
# shifted-window gather/scatter DMA region table
# ======================================================================
def _x_regions(chunk):
    """Regions for loading chunk `chunk` (images b, window-row wi) of x into
    the window-ordered SBUF tile x_t[128, 4, C].

    Token t in [0,512): wj=t//64, pi=(t%64)//8, pj=t%8.
    src pixel: h=(8*wi+4+pi)%64, col=(8*wj+4+pj)%64.
    dst: partition=(wj%2)*64+pi*8+pj, sub=wj//2.
    Returns list of (dst_spec, src_offset_elems, src_dims) where
      dst_spec = (part_lo, part_hi, sub_lo, sub_hi, pj_split or None)
      src_dims = list of [stride_elems, count] matching dst iteration order.
    Strides are relative to x[b] (a [64, 64, C] f32 block).
    """
    b, wi = chunk // 8, chunk % 8
    ROW, COL = W * C, C
    regions = []

    for pi in range(8):
        h = (8 * wi + 4 + pi) % 64
        base = h * ROW
        # even wj in {0,2,4,6}: dst partitions (0 + pi*8 + pj), subs 0..3
        regions.append((
            ("evenodd", 0, pi, 4),
            base + 4 * COL,
            [[COL, 8], [16 * COL, 4], [1, C]],
        ))
        # odd wj in {1,3,5}: dst partitions 64+pi*8+pj, subs 0..2
        regions.append((
            ("evenodd", 64, pi, 3),
            base + 12 * COL,
            [[COL, 8], [16 * COL, 3], [1, C]],
        ))
        # wj == 7 (dst partitions 64:..., sub 3), pj split for col wrap
        regions.append((
            ("wj7", 64, pi, (0, 4)),
            base + 60 * COL,
            [[COL, 4], [1, C]],
        ))
        regions.append((
            ("wj7", 64, pi, (4, 4)),
            base + 0 * COL,
            [[COL, 4], [1, C]],
        ))
    return b, regions


def _dst_ap(x_t, spec):
    """Build the SBUF-side AP matching a region's src iteration order."""
    kind, pbase, pi, arg = spec
    if kind == "evenodd":
        nsub = arg
        # dst iter: (pj partitions, sub, c)
        return x_t[pbase + pi * 8: pbase + pi * 8 + 8, :nsub, :]
    else:
        pj_lo, n_pj = arg
        # dst iter: (pj partitions, c) at sub=3
        return x_t[pbase + pi * 8 + pj_lo: pbase + pi * 8 + pj_lo + n_pj, 3, :]


def _src_ap(x_d, b, off, dims):
    base = x_d[b]
    return bass.AP(base.tensor, base.offset + off, dims)


# ======================================================================
# device program
# ======================================================================
def _build_nc(bias_nz):
    nz1, nzp, nz3, nz4 = bias_nz
    nc = bacc.Bacc()

    x_d = nc.declare_dram_parameter("x", [BPC, H, W, C], F32, isOutput=False)
    w1_d = nc.declare_dram_parameter("w1", [C, 3 * C], BF16, isOutput=False)
    b1_d = nc.declare_dram_parameter("bb1", [1, 3 * C], BF16, isOutput=False)
    wp_d = nc.declare_dram_parameter("wp", [C, C], BF16, isOutput=False)
    bp_d = nc.declare_dram_parameter("bbp", [1, C], BF16, isOutput=False)
    w3_d = nc.declare_dram_parameter("w3", [C, 4 * C], BF16, isOutput=False)
    b3_d = nc.declare_dram_parameter("bb3", [1, 4 * C], BF16, isOutput=False)
    w4_d = nc.declare_dram_parameter("w4", [4 * C, C], BF16, isOutput=False)
    b4_d = nc.declare_dram_parameter("bb4", [1, C], BF16, isOutput=False)
    bm_d = nc.declare_dram_parameter("bm", [PP, 4, NH * PP], BF16, isOutput=False)
    id_d = nc.declare_dram_parameter("ident", [PP, PP], BF16, isOutput=False)
    out_d = nc.declare_dram_parameter("out", [BPC, H, W, C], F32, isOutput=True)

    KC = [(0, 128), (128, 64)]  # C=192 split for feature-major partition chunks

    from contextlib import ExitStack

    with tile.TileContext(nc) as tc, ExitStack() as ctx:
        const = ctx.enter_context(tc.tile_pool(name="const", bufs=1))
        xp = ctx.enter_context(tc.tile_pool(name="xp", bufs=2))
        h1p = ctx.enter_context(tc.tile_pool(name="h1p", bufs=2))
        fmp = ctx.enter_context(tc.tile_pool(name="fmp", bufs=2))
        qkp = ctx.enter_context(tc.tile_pool(name="qkp", bufs=2))
        vdp = ctx.enter_context(tc.tile_pool(name="vdp", bufs=2))
        exq = ctx.enter_context(tc.tile_pool(name="exq", bufs=3))
        atp = ctx.enter_context(tc.tile_pool(name="atp", bufs=2))
        smp = ctx.enter_context(tc.tile_pool(name="smp", bufs=2))
        outp = ctx.enter_context(tc.tile_pool(name="outp", bufs=2))
        ps_big = ctx.enter_context(tc.tile_pool(name="ps_big", bufs=3, space="PSUM"))
        ps_sim = ctx.enter_context(tc.tile_pool(name="ps_sim", bufs=2, space="PSUM"))
        ps_att = ctx.enter_context(tc.tile_pool(name="ps_att", bufs=2, space="PSUM"))
        ps_v = ctx.enter_context(tc.tile_pool(name="ps_v", bufs=1, space="PSUM"))

        # ---- persistent constants
        w1_sb = const.tile([128, 2, 3 * C], BF16)
        wp_sb = const.tile([128, 2, C], BF16)
        w3_sb = const.tile([128, 2, 4 * C], BF16)
        w4_sb = const.tile([128, 6, C], BF16)
        b1_sb = const.tile([1, 3 * C], BF16)
        bp_sb = const.tile([1, C], BF16)
        b3_sb = const.tile([1, 4 * C], BF16)
        b4_sb = const.tile([1, C], BF16)
        bm_sb = const.tile([PP, 4, NH * PP], BF16)
        id_sb = const.tile([PP, PP], BF16)
        ones_sb = const.tile([1, CH], BF16)
        eps_sb = const.tile([128, 1], F32)
        nc.gpsimd.memset(eps_sb[:], EPS)
        # persistent double-buffered block-diag q operand and v operand:
        # zeros/ones regions are written once; data blocks rewritten per chunk
        qbd2 = []
        for _i in range(2):
            _q = const.tile([96, 2, 8, 192], BF16, tag=f"qbd{_i}")
            nc.gpsimd.memset(_q[:], 0.0)
            qbd2.append(_q)
        vbd2 = []
        for _i in range(2):
            row = []
            for _w in range(4):
                _v = const.tile([128, NH, 66], BF16, tag=f"vbd{_i}_{_w}")
                nc.gpsimd.memset(_v[0:64, :, 32:66], 0.0)
                nc.gpsimd.memset(_v[64:128, :, 0:33], 0.0)
                nc.gpsimd.memset(_v[64:128, :, 65], 1.0)
                nc.gpsimd.memset(_v[0:64, :, 32], 1.0)
                row.append(_v)
            vbd2.append(row)

        for kc, (off, sz) in enumerate(KC):
            nc.sync.dma_start(w1_sb[:sz, kc, :], w1_d[off:off + sz, :])
            nc.sync.dma_start(wp_sb[:sz, kc, :], wp_d[off:off + sz, :])
            nc.sync.dma_start(w3_sb[:sz, kc, :], w3_d[off:off + sz, :])
        for kc in range(6):
            nc.sync.dma_start(w4_sb[:, kc, :], w4_d[kc * 128:(kc + 1) * 128, :])
        nc.sync.dma_start(b1_sb[:], b1_d[:])
        nc.sync.dma_start(bp_sb[:], bp_d[:])
        nc.sync.dma_start(b3_sb[:], b3_d[:])
        nc.sync.dma_start(b4_sb[:], b4_d[:])
        nc.sync.dma_start(bm_sb[:], bm_d[:])
        nc.sync.dma_start(id_sb[:], id_d[:])
        nc.gpsimd.memset(ones_sb[:], 1.0)

        def layernorm(x_t, dst_bf16, spool):
            """x_t [128, 4, C] f32 -> dst [128, 4, 256] bf16 (cols :C valid)."""
            st6 = spool.tile([128, 4, 6], F32, tag="st6")
            mv = spool.tile([128, 4, 2], F32, tag="mv")
            sd = spool.tile([128, 4], F32, tag="sd")
            rstd = spool.tile([128, 4], F32, tag="rstd")
            mrs = spool.tile([128, 4], F32, tag="mrs")
            for s in range(4):
                nc.vector.bn_stats(st6[:, s, :], x_t[:, s, :])
                nc.vector.bn_aggr(mv[:, s, :], st6[:, s, :])
            nc.scalar.activation(sd[:], mv[:, :, 1], AF.Sqrt, bias=eps_sb[:])
            nc.vector.reciprocal(rstd[:], sd[:])
            nc.vector.tensor_tensor(mrs[:], mv[:, :, 0], rstd[:], ALU.mult)
            nc.vector.tensor_scalar_mul(mrs[:], mrs[:], -1.0)
            for s in range(4):
                nc.scalar.activation(
                    dst_bf16[:, s, :C], x_t[:, s, :], AF.Identity,
                    bias=mrs[:, s, None], scale=rstd[:, s, None],
                )

        def fm_transpose(src, dst0, dst1):
            """src [128, 4, 256] bf16 token-major -> dst0 [128, CH] (C 0:128),
            dst1 [128, CH] rows 0:64 = C 128:192."""
            nc.gpsimd.memset(src[:, :, C:256], 0.0)
            for s in range(4):
                nc.sync.dma_start(
                    dst0[:, s * 128:(s + 1) * 128], src[:, s, 0:128],
                    transpose=True)
                nc.sync.dma_start(
                    dst1[:, s * 128:(s + 1) * 128], src[:, s, 128:256],
                    transpose=True)

        def copy_eng(i):
            if i % 2 == 0:
                return lambda out, in_: nc.scalar.copy(out, in_)
            return lambda out, in_: nc.vector.tensor_copy(out, in_)

        # ================= main chunk loop =================
        dbg_nch = int(os.environ.get("K_DBG_NCH", NCH))
        dbg_phase = int(os.environ.get("K_DBG_PHASE", 9))
        for c in range(dbg_nch):
            b, regions = _x_regions(c)
            wi = c % 8

            x_t = xp.tile([128, 4, C], F32, tag="x")
            for spec, off, dims in regions:
                nc.sync.dma_start(_dst_ap(x_t, spec), _src_ap(x_d, b, off, dims))

            if dbg_phase < 2:
                continue
            # ---- LN1 -> h1 (token-major bf16, padded to 256)
            h1 = h1p.tile([128, 4, 256], BF16, tag="h1")
            layernorm(x_t, h1, smp)

            h1T0 = fmp.tile([128, CH], BF16, tag="h1T0")
            h1T1 = fmp.tile([128, CH], BF16, tag="h1T1")
            fm_transpose(h1, h1T0, h1T1)
            h1T = [h1T0, h1T1]

            if dbg_phase < 3:
                continue
            # ---- qkv: q/k feature-major [96-chunks, CH]
            qk_sb = qkp.tile([96, 4, CH], BF16, tag="qk")
            for m in range(4):
                psq = ps_big.tile([128, CH], F32, tag="big")
                first = True
                if nz1:
                    nc.tensor.matmul(
                        psq[:96], b1_sb[:, 96 * m:96 * (m + 1)], ones_sb[:],
                        start=True, stop=False, skip_group_check=True)
                    first = False
                for kc, (off, sz) in enumerate(KC):
                    nc.tensor.matmul(
                        psq[:96], w1_sb[:sz, kc, 96 * m:96 * (m + 1)],
                        h1T[kc][:sz, :],
                        start=first, stop=(kc == 1), skip_group_check=True)
                    first = False
                copy_eng(m)(qk_sb[:, m, :], psq[:96])

            if dbg_phase < 4:
                continue
            # ---- v token-major per window-pair + block-diag layout
            v_bds = []
            for wp_i in range(4):
                psv = ps_v.tile([128, C], F32, tag="v")
                first = True
                if nz1:
                    nc.tensor.matmul(
                        psv, ones_sb[:, 0:128], b1_sb[:, 2 * C:3 * C],
                        start=True, stop=False, skip_group_check=True)
                    first = False
                for kc, (off, sz) in enumerate(KC):
                    nc.tensor.matmul(
                        psv, h1T[kc][:sz, 128 * wp_i:128 * (wp_i + 1)],
                        w1_sb[:sz, kc, 2 * C:3 * C],
                        start=first, stop=(kc == 1), skip_group_check=True)
                    first = False
                v_bd = vbd2[c % 2][wp_i]
                nc.vector.tensor_copy(
                    v_bd[0:64, :, 0:32],
                    psv[0:64].rearrange("p (h d) -> p h d", h=NH))
                nc.scalar.copy(
                    v_bd[64:128, :, 33:65],
                    psv[64:128].rearrange("p (h d) -> p h d", h=NH))
                v_bds.append(v_bd)

            if dbg_phase < 5:
                continue
            # ---- windowed attention per window pair
            # block-diag q operand for grouped (3-head) sim matmuls:
            # qbd[32j+d, g, w, 64j+q] = q_{3g+j}[w-token q, d]; off-diag zero
            # (zeros persist in ring buffers, initialized before the loop)
            qbd = qbd2[c % 2]
            for g in range(2):
                for j in range(3):
                    src = qk_sb[32 * j:32 * j + 32, g, :].rearrange(
                        "p (w q) -> p w q", q=PP)
                    dst = qbd[32 * j:32 * j + 32, g, :, 64 * j:64 * j + PP]
                    copy_eng(g * 3 + j)(dst, src)
            for wp_i in range(4):
                pss = ps_sim.tile([128, NH * PP], F32, tag="sim")
                for wloc in range(2):
                    wj = 2 * wp_i + wloc
                    wtype = (1 if wi == 7 else 0) + (2 if wj == 7 else 0)
                    for g in range(2):
                        nc.tensor.matmul(
                            pss[64 * wloc:64 * wloc + 64,
                                192 * g:192 * (g + 1)],
                            qk_sb[0:96, 2 + g, PP * wj:PP * (wj + 1)],
                            qbd[0:96, g, wj, :],
                            start=(g == 0), stop=False, skip_group_check=True,
                            tile_position=(0, 64 * wloc))
                    nc.tensor.matmul(
                        pss[64 * wloc:64 * wloc + 64, :],
                        id_sb[:], bm_sb[:, wtype, :],
                        start=False, stop=True, skip_group_check=True,
                        tile_position=(0, 64 * wloc))

                exp_sb = exq.tile([128, NH * PP], BF16, tag="exp")
                nc.scalar.activation(exp_sb[0:64, :], pss[0:64, :], AF.Exp)
                nc.scalar.activation(exp_sb[64:128, :], pss[64:128, :], AF.Exp)

                psa = ps_att.tile([64, NH * 66], F32, tag="att")
                for h in range(NH):
                    nc.tensor.matmul(
                        psa[:, 66 * h:66 * (h + 1)],
                        exp_sb[:, PP * h:PP * (h + 1)],
                        v_bds[wp_i][:, h, :],
                        start=(h == 0), stop=(h == NH - 1),
                        skip_group_check=True)

                # psa viewed (s, h): [64, s(stride 33), h(stride 66), .]
                rcp = smp.tile([64, 2, NH], F32, tag="rcp")
                den = bass.AP(
                    psa.tensor, psa.offset + 32,
                    [psa.ap[0], [33, 2], [66, NH]])
                nc.vector.reciprocal(rcp[:], den)
                attn_sb = atp.tile([64, 2, 256], BF16, tag="attn")
                nc.gpsimd.memset(attn_sb[:, :, C:256], 0.0)
                unn = bass.AP(
                    psa.tensor, psa.offset,
                    [psa.ap[0], [33, 2], [66, NH], [1, 32]])
                nc.vector.tensor_tensor(
                    attn_sb[:, :, 0:C].rearrange("p s (h d) -> p s h d", h=NH),
                    unn,
                    rcp[:, :, :, None].to_broadcast((64, 2, NH, 32)),
                    ALU.mult)

                # attn -> feature-major attnT
                if wp_i == 0:
                    atT0 = fmp.tile([128, CH], BF16, tag="atT0")
                    atT1 = fmp.tile([128, CH], BF16, tag="atT1")
                for s in range(2):
                    tok0 = (2 * wp_i + s) * 64
                    nc.sync.dma_start(
                        atT0[:, tok0:tok0 + 64], attn_sb[:, s, 0:128],
                        transpose=True)
                    nc.sync.dma_start(
                        atT1[:, tok0:tok0 + 64], attn_sb[:, s, 128:256],
                        transpose=True)

            if dbg_phase < 7:
                continue
            # ---- proj (feature-major out) -> transpose -> token-major
            y1T = fmp.tile([96, 2, CH], BF16, tag="y1T")
            for m in range(2):
                psy = ps_big.tile([128, CH], F32, tag="big")
                first = True
                if nzp:
                    nc.tensor.matmul(
                        psy[:96], bp_sb[:, 96 * m:96 * (m + 1)], ones_sb[:],
                        start=True, stop=False, skip_group_check=True)
                    first = False
                for kc, (off, sz) in enumerate(KC):
                    nc.tensor.matmul(
                        psy[:96], wp_sb[:sz, kc, 96 * m:96 * (m + 1)],
                        (atT0 if kc == 0 else atT1)[:sz, :],
                        start=first, stop=(kc == 1), skip_group_check=True)
                    first = False
                copy_eng(m)(y1T[:, m, :], psy[:96])

            y1_tok = outp.tile([128, 4, C], BF16, tag="y1tok")
            for m in range(2):
                for s in range(4):
                    nc.sync.dma_start(
                        y1_tok[:, s, 96 * m:96 * (m + 1)],
                        y1T[:, m, s * 128:(s + 1) * 128], transpose=True)

            x2 = xp.tile([128, 4, C], F32, tag="x2")
            nc.gpsimd.tensor_tensor(x2[:], x_t[:], y1_tok[:], ALU.add)

            if dbg_phase < 8:
                continue
            # ---- LN2 + MLP
            h2 = h1p.tile([128, 4, 256], BF16, tag="h2")
            layernorm(x2, h2, smp)
            h2T0 = fmp.tile([128, CH], BF16, tag="h2T0")
            h2T1 = fmp.tile([128, CH], BF16, tag="h2T1")
            fm_transpose(h2, h2T0, h2T1)
            h2T = [h2T0, h2T1]

            m_sb = qkp.tile([128, 6, CH], BF16, tag="mid")
            for m in range(6):
                psm = ps_big.tile([128, CH], F32, tag="big")
                first = True
                if nz3:
                    nc.tensor.matmul(
                        psm, b3_sb[:, 128 * m:128 * (m + 1)], ones_sb[:],
                        start=True, stop=False, skip_group_check=True)
                    first = False
                for kc, (off, sz) in enumerate(KC):
                    nc.tensor.matmul(
                        psm, w3_sb[:sz, kc, 128 * m:128 * (m + 1)],
                        h2T[kc][:sz, :],
                        start=first, stop=(kc == 1), skip_group_check=True)
                    first = False
                nc.scalar.activation(m_sb[:, m, :], psm[:], AF.Gelu)

            zT = fmp.tile([96, 2, CH], BF16, tag="zT")
            for m in range(2):
                psz = ps_big.tile([128, CH], F32, tag="big")
                first = True
                if nz4:
                    nc.tensor.matmul(
                        psz[:96], b4_sb[:, 96 * m:96 * (m + 1)], ones_sb[:],
                        start=True, stop=False, skip_group_check=True)
                    first = False
                for kc in range(6):
                    nc.tensor.matmul(
                        psz[:96], w4_sb[:, kc, 96 * m:96 * (m + 1)],
                        m_sb[:, kc, :],
                        start=first, stop=(kc == 5), skip_group_check=True)
                    first = False
                copy_eng(m)(zT[:, m, :], psz[:96])

            z_tok = outp.tile([128, 4, C], BF16, tag="ztok")
            for m in range(2):
                for s in range(4):
                    nc.sync.dma_start(
                        z_tok[:, s, 96 * m:96 * (m + 1)],
                        zT[:, m, s * 128:(s + 1) * 128], transpose=True)

            o_sb = outp.tile([128, 4, C], F32, tag="o")
            nc.gpsimd.tensor_tensor(o_sb[:], x2[:], z_tok[:], ALU.add)

            if dbg_phase < 9:
                continue
            # ---- scatter-store (inverse of the gather)
            for spec, off, dims in regions:
                nc.sync.dma_start(_src_ap(out_d, b, off, dims), _dst_ap(o_sb, spec))

    nc.finalize()
    return nc


# ======================================================================
# public entry point
# ======================================================================
def kernel(**inputs):
    from concourse.bass_utils import run_bass_kernel_spmd

    prep = _host_prep(inputs)
    key = prep["bias_nz"]
    if key not in _CACHE:
        _CACHE[key] = _build_nc(key)
    nc = _CACHE[key]

    x = np.asarray(inputs["x"], np.float32)
    shared = {k: prep[k] for k in
              ("w1", "bb1", "wp", "bbp", "w3", "bb3", "w4", "bb4", "bm", "ident")}
    in_maps = []
    for i in range(NCORES):
        m = dict(shared)
        m["x"] = np.ascontiguousarray(x[i * BPC:(i + 1) * BPC])
        in_maps.append(m)

    res = run_bass_kernel_spmd(nc, in_maps, list(range(NCORES)))
    globals()["LAST_RESULT"] = res
    out = np.empty((B, H, W, C), np.float32)
    for i in range(NCORES):
        out[i * BPC:(i + 1) * BPC] = res.results[i]["out"]
    return out
